# revision 82
# baseline (speedup 1.0000x reference)
"""Trainium2 Bass kernel for nn_DynamicConv.

Math (per token t):
    gen[t, :]  = e[t, :] @ W_weight.T + b_weight          # [4096] per-token conv weights
    w[t]       = gen[t].reshape(C_IN, C_OUT)
    out[t, o]  = sum_i x[t, i] * w[t, i, o] + (e[t] @ W_bias.T + b_bias)[o]

Kernel strategy (8 cores, data-parallel over the batch dim B=8):
  * Heavy compute is the gen matmul [4096 tok, 256] @ [256, 4096] per core.
    Done on TensorE in bf16 (fp32 PSUM accumulate), never materialized to HBM.
  * W columns are permuted o-major (j' = o*64 + i) on the host so that the
    per-token contraction over i reduces the *innermost* 64 elements.
  * ScalarE copies each PSUM tile to SBUF as bf16.
  * VectorE multiplies by x (broadcast over o via a stride-0 AP) and reduces
    over i with a pairwise tree (bf16 keeps the 2x DVE mode). DVE ops span
    GROUP token-tiles to amortize per-op overhead. The pipeline is DVE-bound,
    so no PE warmup: its latency only delays the first PSUM->copy->DVE chain.
  * The dynamic-bias path rides TensorE: e @ W_bias.T accumulated with
    x_ext @ B_ext (B = b_weight.reshape(64,64), ones row adds b_bias, and the
    x@B term is the b_weight contribution to the i-contraction).
  * Output is written bf16 (cast to f32 on host): halves the out DMA and the
    error it adds (~0.1%) is far inside the 2e-2 gate.
  * Resident weights/activations are DMA'd in column chunks ordered so the
    first tile's dependencies land first.
"""

from contextlib import ExitStack

import numpy as np
import ml_dtypes

import concourse.bass as bass
import concourse.tile as tile
from concourse import bacc, mybir
from concourse.bass_utils import run_bass_kernel_spmd

B, N = 8, 4096
C_IN, C_OUT, C_EMB = 64, 64, 256
NUM_W = C_IN * C_OUT  # 4096
P = 128  # tokens per tile (SBUF partitions)
N_TILES_FULL = N // P  # 32 token tiles per core (shard = one batch row)
GROUP = 4  # token tiles per DVE op group
TRB = 256  # tokens per transposed-path block
N_TRB = 2  # transposed blocks (PE-reduced, no DVE tree), interleaved

BF16 = mybir.dt.bfloat16
F32 = mybir.dt.float32
BF16_NP = ml_dtypes.bfloat16
COPY = mybir.ActivationFunctionType.Copy


def build(nc, n_tiles):
    """Emit the per-core program. Token count = n_tiles * 128."""
    t_tot = n_tiles * P
    NW = NUM_W + C_OUT  # 4160: gen columns + dynamic-bias columns
    w2t_d = nc.dram_tensor("w2t", [C_EMB, NW], BF16, kind="ExternalInput")
    et_d = nc.dram_tensor("et", [C_EMB, t_tot], BF16, kind="ExternalInput")
    xb_d = nc.dram_tensor("xb", [P, n_tiles, C_IN], BF16, kind="ExternalInput")
    xte_d = nc.dram_tensor("xte", [C_IN + 1, t_tot], BF16, kind="ExternalInput")
    bex_d = nc.dram_tensor("bex", [C_IN + 1, C_OUT], BF16, kind="ExternalInput")
    s2_d = nc.dram_tensor("s2", [P, 127], BF16, kind="ExternalInput")
    TR_TOK = TRB * N_TRB
    tr0 = t_tot - TR_TOK  # first token handled by the transposed path
    out_d = nc.dram_tensor("out", [tr0, C_OUT], BF16, kind="ExternalOutput")
    out2_d = nc.dram_tensor("out2", [C_OUT, TR_TOK], BF16, kind="ExternalOutput")

    with tile.TileContext(nc) as tc, ExitStack() as ctx:
        const = ctx.enter_context(tc.tile_pool(name="const", bufs=1))
        genp = ctx.enter_context(tc.tile_pool(name="gen", bufs=2))
        tmpp = ctx.enter_context(tc.tile_pool(name="tmp", bufs=2))
        trp = ctx.enter_context(tc.tile_pool(name="trp", bufs=3))
        outp = ctx.enter_context(tc.tile_pool(name="outp", bufs=2))
        psg = ctx.enter_context(tc.tile_pool(name="psg", bufs=3, space="PSUM"))
        psb = ctx.enter_context(tc.tile_pool(name="psb", bufs=1, space="PSUM"))
        pst = ctx.enter_context(tc.tile_pool(name="pst", bufs=1, space="PSUM"))

        # Resident tensors.
        w2t = [const.tile([P, NW], BF16, tag=f"w2t{k}", name=f"w2t{k}") for k in range(2)]
        et = [const.tile([P, t_tot], BF16, tag=f"et{k}", name=f"et{k}") for k in range(2)]
        xb = const.tile([P, n_tiles, C_IN], BF16, tag="xb")
        xte = const.tile([C_IN + 1, t_tot], BF16, tag="xte")
        bex = const.tile([C_IN + 1, C_OUT], BF16, tag="bex")
        # Selection strip: chunk q's [128,64] 0/1 matrix is the slice
        # s2[:, 62-2q : 126-2q] (all 32 matrices are column-shifts).
        s2 = const.tile([P, 127], BF16, tag="s2")
        # x replicated over both o-halves of an io-chunk, for the transposed
        # path's elementwise multiply: xrep[(oh, i), t] = x[t, i].
        xrep = const.tile([P, TR_TOK], BF16, tag="xrep")

        # Load in column chunks, first-tile dependencies first.
        CW = 1024  # chunk width
        wchunks = [(0, CW), (CW, 2 * CW), (2 * CW, 3 * CW), (3 * CW, NW)]
        cwe = min(CW, t_tot)
        n_ec = t_tot // cwe  # chunks for token-indexed tensors
        tpc = n_tiles // n_ec  # tiles per chunk

        for k in range(2):
            nc.sync.dma_start(w2t[k][:, 0:CW], w2t_d[slice(k * P, (k + 1) * P), 0:CW])
        cs0 = slice(0, cwe)
        for k in range(2):
            nc.sync.dma_start(et[k][:, cs0], et_d[slice(k * P, (k + 1) * P), cs0])
        nc.sync.dma_start(xb[:, 0:tpc, :], xb_d[:, 0:tpc, :])
        for lo, hi in wchunks[1:]:
            for k in range(2):
                nc.sync.dma_start(w2t[k][:, lo:hi], w2t_d[slice(k * P, (k + 1) * P), lo:hi])
        nc.sync.dma_start(xte[:, cs0], xte_d[:, cs0])
        nc.sync.dma_start(bex[:], bex_d[:])
        nc.sync.dma_start(s2[:], s2_d[:])
        for c in range(1, n_ec):
            cs = slice(c * cwe, (c + 1) * cwe)
            for k in range(2):
                nc.sync.dma_start(et[k][:, cs], et_d[slice(k * P, (k + 1) * P), cs])
            nc.sync.dma_start(
                xb[:, c * tpc:(c + 1) * tpc, :], xb_d[:, c * tpc:(c + 1) * tpc, :]
            )
            nc.sync.dma_start(xte[:, cs], xte_d[:, cs])

        def build_xrep():
            # Deferred: depends on the LAST xte chunk; emitting it up front
            # would block ScalarE's queue (and thus tile 0's copies) on it.
            nc.scalar.activation(xrep[0:C_IN, :], xte[0:C_IN, tr0:t_tot], COPY)
            nc.scalar.activation(xrep[C_IN:2 * C_IN, :], xte[0:C_IN, tr0:t_tot], COPY)

        def do_group(t0, glen, early_mul=False):
            # gen2[t, o*64+i] = e[t] @ W2T for glen tiles -> bf16 SBUF.
            genb = genp.tile([P, GROUP * NUM_W], BF16, tag="genb")
            tmp1 = tmpp.tile([P, GROUP, C_OUT, C_IN], BF16, tag="t1")
            for u in range(glen):
                ts = bass.ts(t0 + u, P)
                for q in range(4):
                    ps = psg.tile([P, 1024], F32, tag="ps")
                    for k in range(2):
                        nc.tensor.matmul(
                            ps[:, 0:512],
                            et[k][:, ts],
                            w2t[k][:, q * 1024 + 0:q * 1024 + 512],
                            start=(k == 0),
                            stop=(k == 1),
                        )
                        nc.tensor.matmul(
                            ps[:, 512:1024],
                            et[k][:, ts],
                            w2t[k][:, q * 1024 + 512:q * 1024 + 1024],
                            start=(k == 0),
                            stop=(k == 1),
                        )
                    dst = genb[:, u * NUM_W + q * 1024:u * NUM_W + (q + 1) * 1024]
                    nc.scalar.activation(dst, ps[:], COPY)
                    if early_mul:
                        # Pipeline-fill: multiply this q-block as soon as its
                        # copy lands instead of waiting for the whole tile.
                        gq = dst.rearrange("p (o i) -> p o i", i=C_IN)
                        xq = (
                            xb[:, t0 + u, :]
                            .unsqueeze(1)
                            .broadcast_to([P, 16, C_IN])
                        )
                        nc.vector.tensor_mul(
                            tmp1[:, u, 16 * q:16 * (q + 1), :], gq, xq
                        )

            # Dynamic bias: e @ W_bias.T + x @ B + b_bias (ones row of xte).
            pb = psb.tile([P, GROUP * C_OUT], F32, tag="pb")
            for u in range(glen):
                ts = bass.ts(t0 + u, P)
                po = pb[:, u * C_OUT:(u + 1) * C_OUT]
                nc.tensor.matmul(po, et[0][:, ts], w2t[0][:, NUM_W:NW], start=True, stop=False)
                nc.tensor.matmul(po, et[1][:, ts], w2t[1][:, NUM_W:NW], start=False, stop=False)
                nc.tensor.matmul(po, xte[:, ts], bex[:], start=False, stop=True)

            # tmp1[t, u, o, i] = gen2[t, u, o, i] * x[t, u, i]
            t1v = tmp1[:, 0:glen]
            if not early_mul:
                genb4 = genb[:, 0:glen * NUM_W].rearrange(
                    "p (u o i) -> p u o i", u=glen, i=C_IN
                )
                xv = (
                    xb[:, t0:t0 + glen, :]
                    .unsqueeze(2)
                    .broadcast_to([P, glen, C_OUT, C_IN])
                )
                nc.vector.tensor_mul(t1v, genb4, xv)

            # Pairwise tree reduction over i, ping-ponging between tmp1 and
            # a dead region of genb (fully consumed by the mul above): every
            # op has distinct in/out tiles (in-place same-region ops
            # serialize the DVE pipeline) at zero extra SBUF.
            av = genb[:, 0:glen * C_OUT * 32].rearrange(
                "p (u o w) -> p u o w", u=glen, w=32
            )
            src, dst = t1v, av
            w = C_IN // 2
            while w >= 1:
                nc.vector.tensor_add(
                    dst[:, :, :, 0:w], src[:, :, :, 0:w], src[:, :, :, w:2 * w]
                )
                src, dst = dst, src
                w //= 2
            cur = src[:, :, :, 0:1]

            # ScalarE (has slack) lands pb in SBUF bf16 so the final add
            # keeps the 2x DVE mode and frees the PSUM bank early.
            pbs = outp.tile([P, GROUP, C_OUT], BF16, tag="pbs")
            pbv = pb[:, 0:glen * C_OUT].rearrange("p (u o) -> p u o", u=glen)
            nc.scalar.activation(pbs[:, 0:glen], pbv, COPY)
            outs = outp.tile([P, GROUP, C_OUT], BF16, tag="os")
            nc.vector.tensor_add(outs[:, 0:glen], cur[:, :, :, 0], pbs[:, 0:glen])
            dst = out_d[t0 * P:(t0 + glen) * P, :].rearrange(
                "(u p) o -> p u o", u=glen
            )
            nc.sync.dma_start(dst, outs[:, 0:glen])

        # ---- Transposed-path step emitters (for the last TR_TOK tokens).
        # gen^T [io, tok] chunks in PSUM -> scalar copy -> DVE multiply by
        # xrep (no tree) -> PE reduces over i via 0/1 selection matmuls that
        # accumulate onto the dynamic bias in PSUM. Out lands as out^T.
        def tr_bias(bi):
            ts = slice(tr0 + bi * TRB, tr0 + (bi + 1) * TRB)
            pso = pst.tile([P, TRB], F32, tag="pso", name=f"pso{bi}")
            po = pso[0:C_OUT, :]
            nc.tensor.matmul(po, w2t[0][:, NUM_W:NW], et[0][:, ts], start=True, stop=False)
            nc.tensor.matmul(po, w2t[1][:, NUM_W:NW], et[1][:, ts], start=False, stop=False)
            nc.tensor.matmul(po, bex[:], xte[:, ts], start=False, stop=False)
            return pso

        def tr_batch(bi, b4):
            ts = slice(tr0 + bi * TRB, tr0 + (bi + 1) * TRB)
            tsl = slice(bi * TRB, (bi + 1) * TRB)
            psg_t = psg.tile([P, 1024], F32, tag="ps")
            for c in range(4):
                q = 4 * b4 + c
                dst = psg_t[:, c * TRB:(c + 1) * TRB]
                for k in range(2):
                    nc.tensor.matmul(
                        dst,
                        w2t[k][:, q * P:(q + 1) * P],
                        et[k][:, ts],
                        start=(k == 0),
                        stop=(k == 1),
                    )
            gb = trp.tile([P, 4 * TRB], BF16, tag="gT")
            nc.scalar.activation(gb[:], psg_t[:], COPY)
            xv = xrep[:, tsl].unsqueeze(1).broadcast_to([P, 4, TRB])
            tmpt = trp.tile([P, 4 * TRB], BF16, tag="tT")
            nc.vector.tensor_mul(
                tmpt[:].rearrange("p (c t) -> p c t", c=4),
                gb[:].rearrange("p (c t) -> p c t", c=4),
                xv,
            )
            return tmpt

        def tr_smm(pso, b4, tmpt):
            po = pso[0:C_OUT, :]
            for c in range(4):
                q = 4 * b4 + c
                nc.tensor.matmul(
                    po,
                    s2[:, 62 - 2 * q:126 - 2 * q],
                    tmpt[:, c * TRB:(c + 1) * TRB],
                    start=False,
                    stop=(q == 31),
                )

        def tr_out(bi, pso):
            tsl = slice(bi * TRB, (bi + 1) * TRB)
            outt = outp.tile([C_OUT, TRB], BF16, tag="ot")
            nc.scalar.activation(outt[:], pso[0:C_OUT, :], COPY)
            nc.sync.dma_start(out2_d[:, tsl], outt[:])

        # ---- Schedule: normal groups cover the first tr0 tokens; transposed
        # batches are interleaved after the later groups so the PE->Scalar->
        # DVE->PE chain of each batch overlaps a full normal group.
        head = [1, 1, 2, 4, 4]
        mid = tr0 // P - sum(head)
        assert mid % GROUP == 0
        sizes = head + [GROUP] * (mid // GROUP)
        slots = [3, 3, 2, 3, 3, 2]
        assert sum(slots) == 8 * N_TRB
        start_g = len(sizes) - len(slots)
        batch_idx = 0
        psos = [None] * N_TRB
        prev_batches = []
        t0 = 0
        for gi, s in enumerate(sizes):
            do_group(t0, s, early_mul=(gi < 4))
            t0 += s
            if gi == start_g - 1:
                build_xrep()
            if gi < start_g:
                continue
            for bi, b4, tmpt in prev_batches:
                tr_smm(psos[bi], b4, tmpt)
                if b4 == 7:
                    tr_out(bi, psos[bi])
            prev_batches = []
            for _ in range(slots[gi - start_g]):
                bi, b4 = divmod(batch_idx, 8)
                if b4 == 0:
                    psos[bi] = tr_bias(bi)
                prev_batches.append((bi, b4, tr_batch(bi, b4)))
                batch_idx += 1
        for bi, b4, tmpt in prev_batches:
            tr_smm(psos[bi], b4, tmpt)
            if b4 == 7:
                tr_out(bi, psos[bi])
    return out_d


def _prep_core_inputs(x_b, e_b, w2t, bex, s2):
    """Per-core input marshalling: transposes/casts only (no math)."""
    t_tot = x_b.shape[0]
    n_tiles = t_tot // P
    et = np.ascontiguousarray(e_b.T).astype(BF16_NP)
    xb = np.ascontiguousarray(
        x_b.reshape(n_tiles, P, C_IN).transpose(1, 0, 2)
    ).astype(BF16_NP)
    xte = np.concatenate(
        [x_b.T, np.ones((1, t_tot), np.float32)], axis=0
    ).astype(BF16_NP)
    return {"w2t": w2t, "et": et, "xb": xb, "xte": xte, "bex": bex, "s2": s2}


def prep_shared(W_weight, b_weight, W_bias, b_bias):
    # o-major column permutation: W2[o*64+i, c] = W_weight[i*64+o, c],
    # then W_bias.T appended as 64 extra columns (the dynamic-bias path).
    w2 = W_weight.reshape(C_IN, C_OUT, C_EMB).transpose(1, 0, 2).reshape(NUM_W, C_EMB)
    w2t = np.concatenate([w2.T, W_bias.T], axis=1)
    w2t = np.ascontiguousarray(w2t).astype(BF16_NP)
    bex = np.concatenate(
        [b_weight.reshape(C_IN, C_OUT), b_bias.reshape(1, C_OUT)], axis=0
    ).astype(BF16_NP)
    # Selection strip for the transposed tail: chunk q's 0/1 matrix
    # (io row (o-2q)*64+i -> output column o) is s2[:, 62-2q:126-2q].
    s2 = np.zeros((P, 127), np.float32)
    r = np.arange(P)
    s2[r, 62 + r // C_IN] = 1.0
    s2 = s2.astype(BF16_NP)
    return w2t, bex, s2


_CACHE = {}


def _get_nc(n_tiles, num_devices):
    key = (n_tiles, num_devices)
    if key not in _CACHE:
        nc = bacc.Bacc(
            "TRN2", target_bir_lowering=False, debug=False, num_devices=num_devices
        )
        build(nc, n_tiles)
        nc.compile()
        _CACHE[key] = nc
    return _CACHE[key]


def kernel(x, embed_feature, W_weight, b_weight, W_bias, b_bias, _trace=False):
    x = np.asarray(x, np.float32)
    embed_feature = np.asarray(embed_feature, np.float32)
    W_weight = np.asarray(W_weight, np.float32)
    b_weight = np.asarray(b_weight, np.float32)
    W_bias = np.asarray(W_bias, np.float32)
    b_bias = np.asarray(b_bias, np.float32)
    assert x.shape == (B, N, C_IN) and embed_feature.shape == (B, N, C_EMB)
    w2t, bex, s2 = prep_shared(W_weight, b_weight, W_bias, b_bias)
    in_maps = [
        _prep_core_inputs(x[b], embed_feature[b], w2t, bex, s2) for b in range(B)
    ]
    nc = _get_nc(N_TILES_FULL, B)
    res = run_bass_kernel_spmd(
        nc, in_maps, list(range(B)), trace=_trace,
        trace_cores=list(range(B)) if _trace == "all" else None,
    )
    out = np.stack(
        [
            np.concatenate(
                [
                    np.asarray(res.results[b]["out"], np.float32),
                    np.asarray(res.results[b]["out2"], np.float32).T,
                ],
                axis=0,
            )
            for b in range(B)
        ],
        axis=0,
    )
    kernel.last_result = res
    return out.astype(np.float32)


# revision 83
# speedup vs baseline: 1.0991x; 1.0991x over previous
"""Trainium2 Bass kernel for nn_DynamicConv.

Math (per token t):
    gen[t, :]  = e[t, :] @ W_weight.T + b_weight          # [4096] per-token conv weights
    w[t]       = gen[t].reshape(C_IN, C_OUT)
    out[t, o]  = sum_i x[t, i] * w[t, i, o] + (e[t] @ W_bias.T + b_bias)[o]

Kernel strategy (8 cores, data-parallel over the batch dim B=8):
  * Heavy compute is the gen matmul [4096 tok, 256] @ [256, 4096] per core.
    Done on TensorE in bf16 (fp32 PSUM accumulate), never materialized to HBM.
  * W columns are permuted o-major (j' = o*64 + i) on the host so that the
    per-token contraction over i reduces the *innermost* 64 elements.
  * ScalarE copies each PSUM tile to SBUF as bf16.
  * VectorE multiplies by x (broadcast over o via a stride-0 AP) and reduces
    over i with a pairwise tree (bf16 keeps the 2x DVE mode). DVE ops span
    GROUP token-tiles to amortize per-op overhead. The pipeline is DVE-bound,
    so no PE warmup: its latency only delays the first PSUM->copy->DVE chain.
  * The dynamic-bias path rides TensorE: e @ W_bias.T accumulated with
    x_ext @ B_ext (B = b_weight.reshape(64,64), ones row adds b_bias, and the
    x@B term is the b_weight contribution to the i-contraction).
  * Output is written bf16 (cast to f32 on host): halves the out DMA and the
    error it adds (~0.1%) is far inside the 2e-2 gate.
  * Resident weights/activations are DMA'd in column chunks ordered so the
    first tile's dependencies land first.
"""

from contextlib import ExitStack

import numpy as np
import ml_dtypes

import concourse.bass as bass
import concourse.tile as tile
from concourse import bacc, mybir
from concourse.bass_utils import run_bass_kernel_spmd

B, N = 8, 4096
C_IN, C_OUT, C_EMB = 64, 64, 256
NUM_W = C_IN * C_OUT  # 4096
P = 128  # tokens per tile (SBUF partitions)
N_TILES_FULL = N // P  # 32 token tiles per core (shard = one batch row)
GROUP = 3  # token tiles per DVE op group
TRB = 256  # tokens per transposed-path block
N_TRB = 2  # transposed blocks (PE-reduced, no DVE tree), interleaved

BF16 = mybir.dt.bfloat16
F32 = mybir.dt.float32
BF16_NP = ml_dtypes.bfloat16
COPY = mybir.ActivationFunctionType.Copy


def build(nc, n_tiles):
    """Emit the per-core program. Token count = n_tiles * 128."""
    t_tot = n_tiles * P
    NW = NUM_W + C_OUT  # 4160: gen columns + dynamic-bias columns
    w2t_d = nc.dram_tensor("w2t", [C_EMB, NW], BF16, kind="ExternalInput")
    et_d = nc.dram_tensor("et", [C_EMB, t_tot], BF16, kind="ExternalInput")
    xb_d = nc.dram_tensor("xb", [P, n_tiles, C_IN], BF16, kind="ExternalInput")
    xte_d = nc.dram_tensor("xte", [C_IN + 1, t_tot], BF16, kind="ExternalInput")
    bex_d = nc.dram_tensor("bex", [C_IN + 1, C_OUT], BF16, kind="ExternalInput")
    s2_d = nc.dram_tensor("s2", [P, 32 * C_OUT], BF16, kind="ExternalInput")
    TR_TOK = TRB * N_TRB
    tr0 = t_tot - TR_TOK  # first token handled by the transposed path
    out_d = nc.dram_tensor("out", [tr0, C_OUT], BF16, kind="ExternalOutput")
    out2_d = nc.dram_tensor("out2", [C_OUT, TR_TOK], BF16, kind="ExternalOutput")

    with tile.TileContext(nc) as tc, ExitStack() as ctx:
        const = ctx.enter_context(tc.tile_pool(name="const", bufs=1))
        genp = ctx.enter_context(tc.tile_pool(name="gen", bufs=2))
        tmpp = ctx.enter_context(tc.tile_pool(name="tmp", bufs=2))
        trp = ctx.enter_context(tc.tile_pool(name="trp", bufs=3))
        outp = ctx.enter_context(tc.tile_pool(name="outp", bufs=2))
        psg = ctx.enter_context(tc.tile_pool(name="psg", bufs=3, space="PSUM"))
        psb = ctx.enter_context(tc.tile_pool(name="psb", bufs=1, space="PSUM"))
        pst = ctx.enter_context(tc.tile_pool(name="pst", bufs=1, space="PSUM"))

        # Resident tensors.
        w2t = [const.tile([P, NW], BF16, tag=f"w2t{k}", name=f"w2t{k}") for k in range(2)]
        et = [const.tile([P, t_tot], BF16, tag=f"et{k}", name=f"et{k}") for k in range(2)]
        xb = const.tile([P, n_tiles, C_IN], BF16, tag="xb")
        xte = const.tile([C_IN + 1, t_tot], BF16, tag="xte")
        bex = const.tile([C_IN + 1, C_OUT], BF16, tag="bex")
        s2 = const.tile([P, 32 * C_OUT], BF16, tag="s2")
        # x replicated over both o-halves of an io-chunk, for the transposed
        # path's elementwise multiply: xrep[(oh, i), t] = x[t, i].
        xrep = const.tile([P, TR_TOK], BF16, tag="xrep")

        # Load in column chunks, first-tile dependencies first.
        CW = 1024  # chunk width
        wchunks = [(0, CW), (CW, 2 * CW), (2 * CW, 3 * CW), (3 * CW, NW)]
        cwe = min(CW, t_tot)
        n_ec = t_tot // cwe  # chunks for token-indexed tensors
        tpc = n_tiles // n_ec  # tiles per chunk

        for k in range(2):
            nc.sync.dma_start(w2t[k][:, 0:CW], w2t_d[slice(k * P, (k + 1) * P), 0:CW])
        cs0 = slice(0, cwe)
        for k in range(2):
            nc.sync.dma_start(et[k][:, cs0], et_d[slice(k * P, (k + 1) * P), cs0])
        nc.sync.dma_start(xb[:, 0:tpc, :], xb_d[:, 0:tpc, :])
        for lo, hi in wchunks[1:]:
            for k in range(2):
                nc.sync.dma_start(w2t[k][:, lo:hi], w2t_d[slice(k * P, (k + 1) * P), lo:hi])
        nc.sync.dma_start(xte[:, cs0], xte_d[:, cs0])
        nc.sync.dma_start(bex[:], bex_d[:])
        nc.sync.dma_start(s2[:], s2_d[:])
        for c in range(1, n_ec):
            cs = slice(c * cwe, (c + 1) * cwe)
            for k in range(2):
                nc.sync.dma_start(et[k][:, cs], et_d[slice(k * P, (k + 1) * P), cs])
            nc.sync.dma_start(
                xb[:, c * tpc:(c + 1) * tpc, :], xb_d[:, c * tpc:(c + 1) * tpc, :]
            )
            nc.sync.dma_start(xte[:, cs], xte_d[:, cs])

        def build_xrep():
            # Deferred: depends on the LAST xte chunk; emitting it up front
            # would block ScalarE's queue (and thus tile 0's copies) on it.
            nc.scalar.activation(xrep[0:C_IN, :], xte[0:C_IN, tr0:t_tot], COPY)
            nc.scalar.activation(xrep[C_IN:2 * C_IN, :], xte[0:C_IN, tr0:t_tot], COPY)

        def do_group(t0, glen, early_mul=False):
            # gen2[t, o*64+i] = e[t] @ W2T for glen tiles -> bf16 SBUF.
            genb = genp.tile([P, GROUP * NUM_W], BF16, tag="genb")
            tmp1 = tmpp.tile([P, GROUP, C_OUT, C_IN], BF16, tag="t1")
            for u in range(glen):
                ts = bass.ts(t0 + u, P)
                for q in range(4):
                    ps = psg.tile([P, 1024], F32, tag="ps")
                    for k in range(2):
                        nc.tensor.matmul(
                            ps[:, 0:512],
                            et[k][:, ts],
                            w2t[k][:, q * 1024 + 0:q * 1024 + 512],
                            start=(k == 0),
                            stop=(k == 1),
                        )
                        nc.tensor.matmul(
                            ps[:, 512:1024],
                            et[k][:, ts],
                            w2t[k][:, q * 1024 + 512:q * 1024 + 1024],
                            start=(k == 0),
                            stop=(k == 1),
                        )
                    dst = genb[:, u * NUM_W + q * 1024:u * NUM_W + (q + 1) * 1024]
                    nc.scalar.activation(dst, ps[:], COPY)
                    if early_mul:
                        # Pipeline-fill: multiply this q-block as soon as its
                        # copy lands instead of waiting for the whole tile.
                        gq = dst.rearrange("p (o i) -> p o i", i=C_IN)
                        xq = (
                            xb[:, t0 + u, :]
                            .unsqueeze(1)
                            .broadcast_to([P, 16, C_IN])
                        )
                        nc.vector.tensor_mul(
                            tmp1[:, u, 16 * q:16 * (q + 1), :], gq, xq
                        )

            # Dynamic bias: e @ W_bias.T + x @ B + b_bias (ones row of xte).
            pb = psb.tile([P, GROUP * C_OUT], F32, tag="pb")
            for u in range(glen):
                ts = bass.ts(t0 + u, P)
                po = pb[:, u * C_OUT:(u + 1) * C_OUT]
                nc.tensor.matmul(po, et[0][:, ts], w2t[0][:, NUM_W:NW], start=True, stop=False)
                nc.tensor.matmul(po, et[1][:, ts], w2t[1][:, NUM_W:NW], start=False, stop=False)
                nc.tensor.matmul(po, xte[:, ts], bex[:], start=False, stop=True)

            # tmp1[t, u, o, i] = gen2[t, u, o, i] * x[t, u, i]
            t1v = tmp1[:, 0:glen]
            if not early_mul:
                genb4 = genb[:, 0:glen * NUM_W].rearrange(
                    "p (u o i) -> p u o i", u=glen, i=C_IN
                )
                xv = (
                    xb[:, t0:t0 + glen, :]
                    .unsqueeze(2)
                    .broadcast_to([P, glen, C_OUT, C_IN])
                )
                nc.vector.tensor_mul(t1v, genb4, xv)

            # Pairwise tree reduction over i (innermost).
            cur = t1v
            w = C_IN // 2
            while w >= 1:
                nxt = tmpp.tile([P, GROUP, C_OUT, w], BF16, tag=f"tr{w}")
                nv = nxt[:, 0:glen]
                nc.vector.tensor_add(nv, cur[:, :, :, 0:w], cur[:, :, :, w:2 * w])
                cur = nv
                w //= 2
            cur = cur[:, :, :, 0:1]

            # ScalarE (has slack) lands pb in SBUF bf16 so the final add
            # keeps the 2x DVE mode and frees the PSUM bank early.
            pbs = outp.tile([P, GROUP, C_OUT], BF16, tag="pbs")
            pbv = pb[:, 0:glen * C_OUT].rearrange("p (u o) -> p u o", u=glen)
            nc.scalar.activation(pbs[:, 0:glen], pbv, COPY)
            outs = outp.tile([P, GROUP, C_OUT], BF16, tag="os")
            nc.vector.tensor_add(outs[:, 0:glen], cur[:, :, :, 0], pbs[:, 0:glen])
            dst = out_d[t0 * P:(t0 + glen) * P, :].rearrange(
                "(u p) o -> p u o", u=glen
            )
            nc.sync.dma_start(dst, outs[:, 0:glen])

        # ---- Transposed-path step emitters (for the last TR_TOK tokens).
        # gen^T [io, tok] chunks in PSUM -> scalar copy -> DVE multiply by
        # xrep (no tree) -> PE reduces over i via 0/1 selection matmuls that
        # accumulate onto the dynamic bias in PSUM. Out lands as out^T.
        def tr_bias(bi):
            ts = slice(tr0 + bi * TRB, tr0 + (bi + 1) * TRB)
            pso = pst.tile([P, TRB], F32, tag="pso", name=f"pso{bi}")
            po = pso[0:C_OUT, :]
            nc.tensor.matmul(po, w2t[0][:, NUM_W:NW], et[0][:, ts], start=True, stop=False)
            nc.tensor.matmul(po, w2t[1][:, NUM_W:NW], et[1][:, ts], start=False, stop=False)
            nc.tensor.matmul(po, bex[:], xte[:, ts], start=False, stop=False)
            return pso

        def tr_batch(bi, b4):
            ts = slice(tr0 + bi * TRB, tr0 + (bi + 1) * TRB)
            tsl = slice(bi * TRB, (bi + 1) * TRB)
            psg_t = psg.tile([P, 1024], F32, tag="ps")
            for c in range(4):
                q = 4 * b4 + c
                dst = psg_t[:, c * TRB:(c + 1) * TRB]
                for k in range(2):
                    nc.tensor.matmul(
                        dst,
                        w2t[k][:, q * P:(q + 1) * P],
                        et[k][:, ts],
                        start=(k == 0),
                        stop=(k == 1),
                    )
            gb = trp.tile([P, 4 * TRB], BF16, tag="gT")
            nc.scalar.activation(gb[:], psg_t[:], COPY)
            xv = xrep[:, tsl].unsqueeze(1).broadcast_to([P, 4, TRB])
            tmpt = trp.tile([P, 4 * TRB], BF16, tag="tT")
            nc.vector.tensor_mul(
                tmpt[:].rearrange("p (c t) -> p c t", c=4),
                gb[:].rearrange("p (c t) -> p c t", c=4),
                xv,
            )
            return tmpt

        def tr_smm(pso, b4, tmpt):
            po = pso[0:C_OUT, :]
            for c in range(4):
                q = 4 * b4 + c
                nc.tensor.matmul(
                    po,
                    s2[:, q * C_OUT:(q + 1) * C_OUT],
                    tmpt[:, c * TRB:(c + 1) * TRB],
                    start=False,
                    stop=(q == 31),
                )

        def tr_out(bi, pso):
            tsl = slice(bi * TRB, (bi + 1) * TRB)
            outt = outp.tile([C_OUT, TRB], BF16, tag="ot")
            nc.scalar.activation(outt[:], pso[0:C_OUT, :], COPY)
            nc.sync.dma_start(out2_d[:, tsl], outt[:])

        # ---- Schedule: normal groups cover the first tr0 tokens; transposed
        # batches are interleaved after the later groups so the PE->Scalar->
        # DVE->PE chain of each batch overlaps a full normal group.
        head = [1, 1, 2, 3, 3]
        mid = tr0 // P - sum(head)
        assert mid % GROUP == 0
        sizes = head + [GROUP] * (mid // GROUP)
        slots = [3, 3, 2, 3, 2, 2, 1]
        assert sum(slots) == 8 * N_TRB
        start_g = len(sizes) - len(slots)
        batch_idx = 0
        psos = [None] * N_TRB
        prev_batches = []
        t0 = 0
        for gi, s in enumerate(sizes):
            do_group(t0, s, early_mul=(gi < 4))
            t0 += s
            if gi == start_g - 1:
                build_xrep()
            if gi < start_g:
                continue
            for bi, b4, tmpt in prev_batches:
                tr_smm(psos[bi], b4, tmpt)
                if b4 == 7:
                    tr_out(bi, psos[bi])
            prev_batches = []
            for _ in range(slots[gi - start_g]):
                bi, b4 = divmod(batch_idx, 8)
                if b4 == 0:
                    psos[bi] = tr_bias(bi)
                prev_batches.append((bi, b4, tr_batch(bi, b4)))
                batch_idx += 1
        for bi, b4, tmpt in prev_batches:
            tr_smm(psos[bi], b4, tmpt)
            if b4 == 7:
                tr_out(bi, psos[bi])
    return out_d


def _prep_core_inputs(x_b, e_b, w2t, bex, s2):
    """Per-core input marshalling: transposes/casts only (no math)."""
    t_tot = x_b.shape[0]
    n_tiles = t_tot // P
    et = np.ascontiguousarray(e_b.T).astype(BF16_NP)
    xb = np.ascontiguousarray(
        x_b.reshape(n_tiles, P, C_IN).transpose(1, 0, 2)
    ).astype(BF16_NP)
    xte = np.concatenate(
        [x_b.T, np.ones((1, t_tot), np.float32)], axis=0
    ).astype(BF16_NP)
    return {"w2t": w2t, "et": et, "xb": xb, "xte": xte, "bex": bex, "s2": s2}


def prep_shared(W_weight, b_weight, W_bias, b_bias):
    # o-major column permutation: W2[o*64+i, c] = W_weight[i*64+o, c],
    # then W_bias.T appended as 64 extra columns (the dynamic-bias path).
    w2 = W_weight.reshape(C_IN, C_OUT, C_EMB).transpose(1, 0, 2).reshape(NUM_W, C_EMB)
    w2t = np.concatenate([w2.T, W_bias.T], axis=1)
    w2t = np.ascontiguousarray(w2t).astype(BF16_NP)
    bex = np.concatenate(
        [b_weight.reshape(C_IN, C_OUT), b_bias.reshape(1, C_OUT)], axis=0
    ).astype(BF16_NP)
    # Per-chunk selection matrices for the transposed tail: chunk q covers
    # io rows (o, i) with o in {2q, 2q+1}; s2[:, q*64:(q+1)*64] maps row
    # (o-2q)*64+i to output column o.
    s2 = np.zeros((P, 32 * C_OUT), np.float32)
    r = np.arange(P)
    for q in range(32):
        s2[r, q * C_OUT + 2 * q + r // C_IN] = 1.0
    s2 = s2.astype(BF16_NP)
    return w2t, bex, s2


_CACHE = {}


def _get_nc(n_tiles, num_devices):
    key = (n_tiles, num_devices)
    if key not in _CACHE:
        nc = bacc.Bacc(
            "TRN2", target_bir_lowering=False, debug=False, num_devices=num_devices
        )
        build(nc, n_tiles)
        nc.compile()
        _CACHE[key] = nc
    return _CACHE[key]


def kernel(x, embed_feature, W_weight, b_weight, W_bias, b_bias, _trace=False):
    x = np.asarray(x, np.float32)
    embed_feature = np.asarray(embed_feature, np.float32)
    W_weight = np.asarray(W_weight, np.float32)
    b_weight = np.asarray(b_weight, np.float32)
    W_bias = np.asarray(W_bias, np.float32)
    b_bias = np.asarray(b_bias, np.float32)
    assert x.shape == (B, N, C_IN) and embed_feature.shape == (B, N, C_EMB)
    w2t, bex, s2 = prep_shared(W_weight, b_weight, W_bias, b_bias)
    in_maps = [
        _prep_core_inputs(x[b], embed_feature[b], w2t, bex, s2) for b in range(B)
    ]
    nc = _get_nc(N_TILES_FULL, B)
    res = run_bass_kernel_spmd(
        nc, in_maps, list(range(B)), trace=_trace,
        trace_cores=list(range(B)) if _trace == "all" else None,
    )
    out = np.stack(
        [
            np.concatenate(
                [
                    np.asarray(res.results[b]["out"], np.float32),
                    np.asarray(res.results[b]["out2"], np.float32).T,
                ],
                axis=0,
            )
            for b in range(B)
        ],
        axis=0,
    )
    kernel.last_result = res
    return out.astype(np.float32)
